# revision 7
# baseline (speedup 1.0000x reference)
"""Trainium2 Bass kernel for nn_EntityEmbedding (embedding lookup + mean pool).

reference:  out = mean(weights[x], axis=1);  x [16384,50] int, w [1e6,64] f32.

Data-parallel over 8 cores (2048 batch rows each). A naive implementation
issues one indirect DMA per 128 gathered rows = 800 serial Pool-engine SWDGE
instructions per core (~1.7us each of descriptor-generation = 1.38ms). This
kernel instead uses ~40 large SWDGE dma_gather instructions, whose
int16-index limitation (reach 32768 rows) is handled by two hops:

  hop1: 32 dma_gathers, one per 31250-row vocab bucket. The host pre-buckets
        each core's 102400 tokens by (bucket c, batch-quarter g) and emits
        int16 window-local indices (idx-0 mid-padding up to a uniform
        per-(g,c) capacity C). Gathered rows stream SBUF -> DRAM scratch S,
        laid out quarter-major: S row = g*32C + c*C + k.
  hop2: 8 dma_gathers (two per batch quarter of 512 rows / 25600 tokens).
        In-region S positions fit int16 (32C <= 32768). The destination
        stream is slot-major (d = j*512 + b_local) so each output row's 50
        tokens share one SBUF partition; a DVE strided tensor_reduce then
        yields the mean directly (the table is prescaled by 1/50 on the
        host), one strided store per quarter.

S-writes go through the Activation/SP HWDGE engines so the Pool engine only
runs gather desc-gen. All transfers are plain gathers/copies - no
scatter-add RMW races. Each semaphore is updated from a single SWDGE queue
(ucode shadow-sem constraint) and every wait threshold counts only updates
ordered before the waited-for event (DMA completions may reorder across
in-flight transfers). Capacity C is chosen from the actual data at compile
time (program cached per C); pathological inputs (C > 1024, i.e. >1024 of
102400 tokens of one core in the same (vocab-bucket, batch-quarter) group)
fall back to a baseline indirect-DMA kernel that handles any distribution.
"""
from contextlib import ExitStack

import numpy as np

import concourse.bass as bass
import concourse.bacc as bacc
import concourse.mybir as mybir
from concourse.bass_utils import run_bass_kernel_spmd
from concourse.library_config import mlp

P = 128
B_FULL = 16384
I = 50
H = 64
V = 1000000
N_CORES = 8
B_CORE = B_FULL // N_CORES     # 2048
NBKT = 32
WIN = V // NBKT                # 31250 rows per vocab bucket
NQ = 4                         # batch quarters per core
QROWS = B_CORE // NQ           # 512
QTOK = QROWS * I               # 25600 tokens per quarter
NB_TILES = 4                   # hop1 bucket-tile ring depth
NUM_QUEUES = 1                 # SWDGE queues (fake_nrt may support only 1)


def wrap16(a):
    """logical [n] -> ucode idx layout [16, n/16] (idx i at [i%16, i//16])"""
    n = a.shape[0]
    return np.ascontiguousarray(a.reshape(n // 16, 16).T)


def prep_core(x_core):
    """x_core [2048, 50] int32 -> per-token bucketing metadata + capacity C."""
    v = x_core.reshape(-1).astype(np.int64)          # 102400 tokens
    t = np.arange(v.size)
    b = t // I
    j = t - b * I
    g = b // QROWS                                   # quarter 0..3
    c = v // WIN                                     # bucket 0..31
    lv = (v - c * WIN).astype(np.int16)
    key = (c * NQ + g)
    # ascending vocab within each group -> hop1 HBM reads are sorted within
    # their 8MB window (better row-buffer locality than token order)
    order = np.lexsort((lv, key))
    counts = np.bincount(key[order], minlength=NBKT * NQ)
    cmax = int(counts.max())
    C = ((cmax + P - 1) // P) * P
    starts = np.zeros(NBKT * NQ, dtype=np.int64)
    starts[1:] = np.cumsum(counts)[:-1]
    rank = np.empty(v.size, dtype=np.int64)
    rank[order] = np.arange(v.size) - starts[key[order]]
    return lv, c, g, b, j, rank, C


def finish_core(lv, c, g, b, j, rank, C):
    """Build the wrapped idx arrays for capacity C."""
    A1 = np.zeros((NBKT, NQ * C), dtype=np.int16)    # idx-0 pads
    A1[c, g * C + rank] = lv
    # per-instruction wrap: [NBKT, W, 16] -> [16, NBKT*W]
    W1 = NQ * C // 16
    hidx1 = np.ascontiguousarray(
        A1.reshape(NBKT, W1, 16).transpose(2, 0, 1).reshape(16, NBKT * W1)
    )
    # hop2: dest d = j*QROWS + (b - g*QROWS) within quarter g; S-relative
    # index = c*C + k (region base g*32C)
    A2 = np.zeros((NQ, QTOK), dtype=np.int16)
    d = j * QROWS + (b - g * QROWS)
    A2[g, d] = (c * C + rank).astype(np.int16)
    W2 = QTOK // 16
    hidx2 = np.ascontiguousarray(
        A2.reshape(NQ, W2, 16).transpose(2, 0, 1).reshape(16, NQ * W2)
    )
    return hidx1, hidx2


_NC_CACHE = {}


def build_fast_nc(C):
    if C in _NC_CACHE:
        return _NC_CACHE[C]
    CS = C // P                     # slots per (g,c) subgroup
    NIDX1 = NQ * C                  # static idxs per hop1 instruction
    W1 = NIDX1 // 16                # wrapped width per hop1 instruction
    SREG = NBKT * C                 # S rows per quarter region
    assert SREG <= 32768
    W2 = QTOK // 16                 # 1600
    H2SLOTS = QTOK // P             # 200

    nc = bacc.Bacc(None, num_swdge_queues=NUM_QUEUES)
    w = nc.declare_dram_parameter("weights", [V, H], mybir.dt.float32, isOutput=False)
    hidx1 = nc.declare_dram_parameter(
        "hidx1", [P, NBKT * W1], mybir.dt.int16, isOutput=False
    )
    hidx2 = nc.declare_dram_parameter(
        "hidx2", [P, NQ * W2], mybir.dt.int16, isOutput=False
    )
    out = nc.declare_dram_parameter("out", [B_CORE, H], mybir.dt.float32, isOutput=True)
    s_dram = nc.declare_dram_parameter(
        "sscratch", [NQ * SREG, H], mybir.dt.float32, isOutput=False
    )

    with (
        nc.sbuf_tensor([P, NBKT * W1], mybir.dt.int16) as idx1_sb,
        nc.sbuf_tensor([P, NQ * W2], mybir.dt.int16) as idx2_sb,
        nc.sbuf_tensor([P, NB_TILES * NQ * CS * H], mybir.dt.float32) as btiles,
        nc.sbuf_tensor([P, 2 * H2SLOTS * H], mybir.dt.float32) as h2tiles,
        nc.sbuf_tensor([P, NQ * NQ * H], mybir.dt.float32) as obig,
        ExitStack() as stack,
        nc.Block() as block,
    ):
        # Per-ring-slot sems: every wait threshold equals the cumulative
        # count of updates that are ORDERED before the waited-for event, so
        # out-of-order DMA completions across in-flight transfers can't
        # satisfy a threshold early. Each sem is updated from one SWDGE
        # queue only (hop1 gather c -> queue c%4 -> gs[c%4]).
        sem = lambda n: stack.enter_context(nc.semaphore(n))  # noqa: E731
        isem = sem("isem")
        gs = [sem(f"gs{k}") for k in range(NB_TILES)]   # hop1 gather, slot c%4
        wA = [sem(f"wA{k}") for k in range(NB_TILES)]   # scalar S-writes, slot
        wS = [sem(f"wS{k}") for k in range(NB_TILES)]   # sync S-writes, slot
        h2s = [sem(f"h2s{k}") for k in range(2)]        # hop2 gather, slot g%2
        dsem = sem("dsem")
        osem = sem("osem")
        NROUND = NBKT // NB_TILES          # buckets per ring slot
        BT = NQ * CS * H            # floats per partition per bucket tile

        def btile(k):
            return btiles[:, (k % NB_TILES) * BT:(k % NB_TILES + 1) * BT]

        def h2tile(k):
            return h2tiles[:, (k % 2) * H2SLOTS * H:((k % 2) + 1) * H2SLOTS * H]

        @block.gpsimd
        def _(gpsimd):
            gpsimd.load_library(mlp)
            gpsimd.dma_start(idx1_sb[:], hidx1[:]).then_inc(isem, 16)
            gpsimd.dma_start(idx2_sb[:], hidx2[:]).then_inc(isem, 16)
            gpsimd.wait_ge(isem, 32)
            for c in range(NBKT):
                k, r = c % NB_TILES, c // NB_TILES
                if r >= 1:
                    # slot free once bucket c-NB_TILES fully written to S
                    gpsimd.wait_ge(wA[k], 32 * r)
                    gpsimd.wait_ge(wS[k], 32 * r)
                gpsimd.dma_gather(
                    btile(c).rearrange("p (s h) -> p s h", h=H),
                    w[c * WIN:(c + 1) * WIN],
                    idx1_sb[:, c * W1:(c + 1) * W1],
                    NIDX1,
                    NIDX1,
                    H,
                    queue_num=k % NUM_QUEUES,
                ).then_inc(gs[k], 16)
            # all S-writes complete before any hop2 gather
            for k in range(NB_TILES):
                gpsimd.wait_ge(wA[k], 32 * NROUND)
                gpsimd.wait_ge(wS[k], 32 * NROUND)
            # hop2 split into half-quarters: 801 descs/ring fits the carveout
            HT = QTOK // 2                 # 12800 idxs per half
            for g in range(NQ):
                if g >= 2:
                    gpsimd.wait_ge(dsem, g - 1)
                for half in range(2):
                    gpsimd.dma_gather(
                        h2tile(g)[:, half * (H2SLOTS // 2) * H:
                                  (half + 1) * (H2SLOTS // 2) * H].rearrange(
                            "p (s h) -> p s h", h=H
                        ),
                        s_dram[g * SREG:(g + 1) * SREG],
                        idx2_sb[:, g * W2 + half * (HT // 16):
                                g * W2 + (half + 1) * (HT // 16)],
                        HT,
                        HT,
                        H,
                        # h2s[g%2] is locked to one SWDGE queue -> queue g%2
                        queue_num=(g % 2) % NUM_QUEUES,
                    ).then_inc(h2s[g % 2], 16)

        @block.scalar
        def _(scalar):
            for c in range(NBKT):
                k, r = c % NB_TILES, c // NB_TILES
                scalar.wait_ge(gs[k], 16 * (r + 1))
                for g in (0, 1):
                    scalar.dma_start(
                        s_dram[g * SREG + c * C: g * SREG + (c + 1) * C].rearrange(
                            "(s p) h -> p s h", p=P
                        ),
                        btile(c)[:, g * CS * H:(g + 1) * CS * H].rearrange(
                            "p (s h) -> p s h", h=H
                        ),
                    ).then_inc(wA[k], 16)

        @block.sync
        def _(sync):
            for c in range(NBKT):
                k, r = c % NB_TILES, c // NB_TILES
                sync.wait_ge(gs[k], 16 * (r + 1))
                for g in (2, 3):
                    sync.dma_start(
                        s_dram[g * SREG + c * C: g * SREG + (c + 1) * C].rearrange(
                            "(s p) h -> p s h", p=P
                        ),
                        btile(c)[:, g * CS * H:(g + 1) * CS * H].rearrange(
                            "p (s h) -> p s h", h=H
                        ),
                    ).then_inc(wS[k], 16)
            for g in range(NQ):
                sync.wait_ge(dsem, g + 1)
                sync.dma_start(
                    out[g * QROWS:(g + 1) * QROWS].rearrange("(x p) h -> p x h", p=P),
                    obig[:, g * NQ * H:(g + 1) * NQ * H].rearrange(
                        "p (x h) -> p x h", h=H
                    ),
                ).then_inc(osem, 16)
            sync.wait_ge(osem, 16 * NQ)

        @block.vector
        def _(vector):
            for g in range(NQ):
                vector.wait_ge(h2s[g % 2], 32 * (g // 2 + 1))
                # token d at (partition d%128, slot d//128); d = j*512+b_l
                # -> partition b_l%128, slot 4j + b_l//128. Free layout
                # (j:50, bh:256); reduce over j. Table is prescaled by 1/I
                # on the host, so the reduce directly yields the mean.
                g3 = h2tile(g).rearrange("p (j bh) -> p bh j", j=I)
                nc.vector.tensor_reduce(
                    obig[:, g * NQ * H:(g + 1) * NQ * H], g3,
                    axis=mybir.AxisListType.X, op=mybir.AluOpType.add,
                ).then_inc(dsem, 1)

    nc.compile()
    _NC_CACHE[C] = nc
    return nc


def make_in_maps(x, weights):
    """Host prep. Returns (in_maps, C); in_maps is None if infeasible."""
    x = np.ascontiguousarray(np.asarray(x), dtype=np.int32)
    weights = np.asarray(weights)
    preps = [prep_core(x[cc * B_CORE:(cc + 1) * B_CORE]) for cc in range(N_CORES)]
    C = max(p[-1] for p in preps)
    if C > 1024:
        return None, C
    # prescale so the on-chip reduce yields the mean directly
    ws = np.ascontiguousarray(weights.astype(np.float32) * np.float32(1.0 / I))
    in_maps = []
    for cc in range(N_CORES):
        lv, c, g, b, j, rank, _ = preps[cc]
        h1, h2 = finish_core(lv, c, g, b, j, rank, C)
        in_maps.append({
            "weights": ws,
            "hidx1": np.tile(h1, (8, 1)),
            "hidx2": np.tile(h2, (8, 1)),
            "sscratch": np.zeros((NQ * NBKT * C, H), dtype=np.float32),
        })
    return in_maps, C


def run_fast(x, weights):
    in_maps, C = make_in_maps(x, weights)
    if in_maps is None:
        return None
    nc = build_fast_nc(C)
    res = run_bass_kernel_spmd(nc, in_maps, list(range(N_CORES)))
    return np.concatenate(
        [res.results[cc]["out"] for cc in range(N_CORES)], axis=0
    )


# ---------------------------------------------------------------------------
# Baseline fallback: 800 indirect DMAs per core, handles any distribution.
# ---------------------------------------------------------------------------

NT = B_CORE // P           # 16 batch tiles per core
G_BUFS = 4                 # gathered-tile ring


def _build_baseline_nc():
    if "baseline" in _NC_CACHE:
        return _NC_CACHE["baseline"]
    nc = bacc.Bacc(None)
    x = nc.declare_dram_parameter("x", [B_CORE, I], mybir.dt.int32, isOutput=False)
    w = nc.declare_dram_parameter("weights", [V, H], mybir.dt.float32, isOutput=False)
    out = nc.declare_dram_parameter("out", [B_CORE, H], mybir.dt.float32, isOutput=True)

    TILE_F = I * H
    with (
        nc.sbuf_tensor([P, NT * I], mybir.dt.int32) as idx_sb,
        nc.sbuf_tensor([P, G_BUFS * TILE_F], mybir.dt.float32) as g_sb,
        nc.sbuf_tensor([P, H], mybir.dt.float32) as s_sb,
        nc.sbuf_tensor([P, NT * H], mybir.dt.float32) as obig,
        nc.semaphore("dma") as dma,
        nc.semaphore("dvs") as dvs,
        nc.Block() as block,
    ):
        @block.gpsimd
        def _(gpsimd):
            gpsimd.dma_start(
                idx_sb[:].rearrange("p (t i) -> p t i", t=NT),
                x[:].rearrange("(t p) i -> p t i", p=P),
            ).then_inc(dma, 16)
            gpsimd.wait_ge(dma, 16)
            for t in range(NT):
                if t >= G_BUFS:
                    gpsimd.wait_ge(dvs, t - G_BUFS + 1)
                base = (t % G_BUFS) * TILE_F
                for j in range(I):
                    gpsimd.indirect_dma_start(
                        out=g_sb[:, base + j * H: base + (j + 1) * H],
                        out_offset=None,
                        in_=w[:],
                        in_offset=bass.IndirectOffsetOnAxis(
                            ap=idx_sb[:, t * I + j: t * I + j + 1], axis=0
                        ),
                    ).then_inc(dma, 16)
            gpsimd.wait_ge(dvs, NT)
            gpsimd.dma_start(
                out[:].rearrange("(t p) h -> p t h", p=P),
                obig[:].rearrange("p (t h) -> p t h", t=NT),
            ).then_inc(dma, 16)
            gpsimd.wait_ge(dma, 16 * (1 + NT * I + 1))

        @block.vector
        def _(vector):
            for t in range(NT):
                vector.wait_ge(dma, 16 * (1 + I * (t + 1)))
                base = (t % G_BUFS) * TILE_F
                g3 = g_sb[:, base: base + TILE_F].rearrange(
                    "p (i h) -> p h i", i=I
                )
                nc.vector.tensor_reduce(
                    s_sb[:], g3, axis=mybir.AxisListType.X, op=mybir.AluOpType.add
                )
                nc.vector.tensor_scalar_mul(
                    obig[:, t * H: (t + 1) * H], s_sb[:], 1.0 / I
                ).then_inc(dvs, 1)

    nc.compile()
    _NC_CACHE["baseline"] = nc
    return nc


def _run_baseline(x, weights):
    x = np.ascontiguousarray(np.asarray(x), dtype=np.int32)
    weights = np.ascontiguousarray(np.asarray(weights), dtype=np.float32)
    nc = _build_baseline_nc()
    in_maps = [
        {"x": x[c * B_CORE:(c + 1) * B_CORE], "weights": weights}
        for c in range(N_CORES)
    ]
    res = run_bass_kernel_spmd(nc, in_maps, list(range(N_CORES)))
    return np.concatenate([res.results[c]["out"] for c in range(N_CORES)], axis=0)


# The dma_gather fast path passes CoreSim (incl. race detection) but still
# hits an opaque runtime INTERNAL error on this axon HW stack; keep it
# disabled until that is root-caused. The baseline indirect-DMA path is the
# verified-correct production path.
FAST_ENABLED = False


def kernel(x, weights):
    if FAST_ENABLED:
        try:
            out = run_fast(x, weights)
            if out is not None:
                return out
        except Exception:
            pass
    return _run_baseline(x, weights)
